# revision 9
# baseline (speedup 1.0000x reference)
"""Trainium2 Bass kernel for an 8-layer GPT-style bigram LM (B=4, T=1024,
D=1024, H=16, FF=4096, V=32000), distributed over 8 NeuronCores.

Sharding: tokens are split (batch b, sequence half) -> 8 shards of 512
tokens; core c owns batch c//2, half c%2.  Each core keeps its tokens'
residual stream (feature-major, fp32) for all layers.  Per layer, the
LayerNorm'd activations are exchanged within each (even, odd) core pair
with one AllGather (bf16, 1 MB), after which every core computes K/V for
the full 1024-token sequence locally, and Q/attention/out-proj/FFN only
for its own 512 tokens.  The LM head is token-sharded: each core computes
its 512 tokens x full 32000-vocab logits.  Matmuls run in bf16 with fp32
PSUM accumulation.  Embedding gather, loss reduction (log-softmax + CE
from the returned fp32 logits), and shard assembly happen on the host.

All activations on-device are feature-major ([d, token]) so no PE
transposes are needed anywhere; per-token LayerNorm/softmax statistics are
broadcast across partitions with K=1 matmuls against a ones vector.
Causal masking is data-driven (per-core 0/1 bf16 masks multiply the
exp'd scores) so all 8 cores run one identical SPMD program.
"""

import os
import functools
import numpy as np
import ml_dtypes

os.environ.setdefault("JAX_COMPILATION_CACHE_DIR", "/tmp/jaxcache_bigram")

BF = ml_dtypes.bfloat16

L = 8
D = 1024
T = 1024
H = 16
HS = 64
B = 4
FF = 4096
V = 32000
EPS = 1e-5
NCORES = 8
TOWN = 512  # tokens owned per core
DC = D // 128  # 8 d-chunks
FQ = 4  # ff quarters
SCALE = float(D) ** -0.5

# vocab n-tiles: 62 x 512 + 1 x 256
VTILES = [(j * 512, 512) for j in range(62)] + [(62 * 512, 256)]


@functools.lru_cache(maxsize=1)
def _build_program():
    import concourse.mybir as mybir
    import concourse.tile as tile
    from concourse import bacc

    F32 = mybir.dt.float32
    BF16 = mybir.dt.bfloat16
    AF = mybir.ActivationFunctionType
    OP = mybir.AluOpType

    nc = bacc.Bacc("TRN2", target_bir_lowering=False, debug=False,
                   num_devices=NCORES)

    # ---- DRAM I/O ----
    d_x0 = nc.dram_tensor("x0", [DC, 128, TOWN], F32, kind="ExternalInput").ap()
    d_mask = nc.dram_tensor("mask", [8, 128, TOWN], BF16, kind="ExternalInput").ap()
    d_wq = nc.dram_tensor("wq", [L, DC, 128, D], BF16, kind="ExternalInput").ap()
    d_wk = nc.dram_tensor("wk", [L, DC, 128, D], BF16, kind="ExternalInput").ap()
    d_wv = nc.dram_tensor("wv", [L, DC, 128, D], BF16, kind="ExternalInput").ap()
    d_wo = nc.dram_tensor("wo", [L, DC, 128, D], BF16, kind="ExternalInput").ap()
    d_w1 = nc.dram_tensor("w1", [L, DC, 128, FF], BF16, kind="ExternalInput").ap()
    d_w2 = nc.dram_tensor("w2", [L, FF // 128, 128, D], BF16, kind="ExternalInput").ap()
    d_lmw = nc.dram_tensor("lmw", [DC, 128, V], BF16, kind="ExternalInput").ap()
    d_ln1g = nc.dram_tensor("ln1g", [L, DC, 128], F32, kind="ExternalInput").ap()
    d_ln1b = nc.dram_tensor("ln1b", [L, DC, 128], F32, kind="ExternalInput").ap()
    d_ln2g = nc.dram_tensor("ln2g", [L, DC, 128], F32, kind="ExternalInput").ap()
    d_ln2b = nc.dram_tensor("ln2b", [L, DC, 128], F32, kind="ExternalInput").ap()
    d_bo = nc.dram_tensor("bo", [L, DC, 128], F32, kind="ExternalInput").ap()
    d_b2 = nc.dram_tensor("b2", [L, DC, 128], F32, kind="ExternalInput").ap()
    d_b1 = nc.dram_tensor("b1", [L, FF // 128, 128], F32, kind="ExternalInput").ap()
    d_lnfg = nc.dram_tensor("lnfg", [DC, 128], F32, kind="ExternalInput").ap()
    d_lnfb = nc.dram_tensor("lnfb", [DC, 128], F32, kind="ExternalInput").ap()
    d_logits = nc.dram_tensor("logits", [TOWN // 128, 128, V], F32,
                              kind="ExternalOutput").ap()

    # per-layer collective bounce buffers
    d_ccin = [nc.dram_tensor(f"ccin{l}", [DC, 128, TOWN], BF16) for l in range(L)]
    d_ccout = [nc.dram_tensor(f"ccout{l}", [2, DC, 128, TOWN], BF16) for l in range(L)]
    groups = [[0, 1], [2, 3], [4, 5], [6, 7]]

    import contextlib
    with tile.TileContext(nc) as tc, \
         nc.allow_low_precision(reason="bf16 network by design; errors bounded"):
        ctx = contextlib.ExitStack()
        const = ctx.enter_context(tc.tile_pool(name="const", bufs=1))
        state = ctx.enter_context(tc.tile_pool(name="state", bufs=1))
        acts = ctx.enter_context(tc.tile_pool(name="acts", bufs=1))
        wpool = ctx.enter_context(tc.tile_pool(name="wpool", bufs=3))
        tmp = ctx.enter_context(tc.tile_pool(name="tmp", bufs=3))
        wsb = ctx.enter_context(tc.tile_pool(name="wsb", bufs=3))
        stg = ctx.enter_context(tc.tile_pool(name="stg", bufs=4))
        pp_mm = ctx.enter_context(tc.tile_pool(name="pp_mm", bufs=3, space="PSUM"))
        pp_att = ctx.enter_context(tc.tile_pool(name="pp_att", bufs=3, space="PSUM"))
        pp_row = ctx.enter_context(tc.tile_pool(name="pp_row", bufs=2, space="PSUM"))

        # ---- constants ----
        ones_col_bf = const.tile([128, 1], BF16)
        nc.vector.memset(ones_col_bf, 1.0)
        ones_row_bf = const.tile([1, 128], BF16)
        nc.vector.memset(ones_row_bf, 1.0)
        eps_sb = const.tile([1, 1], F32)
        nc.vector.memset(eps_sb, EPS)

        sb_ln1g = const.tile([128, L, DC], F32)
        nc.sync.dma_start(out=sb_ln1g, in_=d_ln1g.rearrange("l c p -> p l c"))
        sb_ln1b = const.tile([128, L, DC], F32)
        nc.sync.dma_start(out=sb_ln1b, in_=d_ln1b.rearrange("l c p -> p l c"))
        sb_ln2g = const.tile([128, L, DC], F32)
        nc.sync.dma_start(out=sb_ln2g, in_=d_ln2g.rearrange("l c p -> p l c"))
        sb_ln2b = const.tile([128, L, DC], F32)
        nc.sync.dma_start(out=sb_ln2b, in_=d_ln2b.rearrange("l c p -> p l c"))
        sb_bo = const.tile([128, L, DC], F32)
        nc.sync.dma_start(out=sb_bo, in_=d_bo.rearrange("l c p -> p l c"))
        sb_b2 = const.tile([128, L, DC], F32)
        nc.sync.dma_start(out=sb_b2, in_=d_b2.rearrange("l c p -> p l c"))
        sb_b1 = const.tile([128, L, FF // 128], F32)
        nc.sync.dma_start(out=sb_b1, in_=d_b1.rearrange("l c p -> p l c"))
        sb_lnfg = const.tile([128, DC], F32)
        nc.sync.dma_start(out=sb_lnfg, in_=d_lnfg.rearrange("c p -> p c"))
        sb_lnfb = const.tile([128, DC], F32)
        nc.sync.dma_start(out=sb_lnfb, in_=d_lnfb.rearrange("c p -> p c"))

        sb_mask = const.tile([128, 8, TOWN], BF16)
        nc.sync.dma_start(out=sb_mask, in_=d_mask.rearrange("c p t -> p c t"))

        x_sb = state.tile([128, DC, TOWN], F32)
        nc.sync.dma_start(out=x_sb, in_=d_x0.rearrange("c p t -> p c t"))

        def layernorm(g_ap, b_ap, hn_out):
            """Feature-major LN of x_sb -> hn_out (bf16 [128, DC, TOWN])."""
            # mean over d (partitions x chunks) via ones-matmul colsums
            ps_mean = pp_row.tile([1, TOWN], F32, tag="row")
            xbf_list = []
            for c in range(DC):
                xbf = tmp.tile([128, TOWN], BF16, tag="xbf")
                nc.scalar.copy(xbf, x_sb[:, c, :])
                xbf_list.append(xbf)
                nc.tensor.matmul(ps_mean, ones_col_bf, xbf,
                                 start=(c == 0), stop=(c == DC - 1))
            m_sb = tmp.tile([1, TOWN], BF16, tag="mrow")
            nc.scalar.mul(m_sb, ps_mean, 1.0 / D)
            ps_mb = pp_mm.tile([128, TOWN], F32, tag="mm")
            nc.tensor.matmul(ps_mb, ones_row_bf, m_sb, start=True, stop=True)
            # t = x - mean (bf16), var = colsum(t^2)/D
            t_t = acts.tile([128, DC, TOWN], BF16, tag="t")
            ps_var = pp_row.tile([1, TOWN], F32, tag="row")
            for c in range(DC):
                nc.vector.tensor_tensor(out=t_t[:, c, :], in0=x_sb[:, c, :],
                                        in1=ps_mb, op=mybir.AluOpType.subtract)
            for c in range(DC):
                t2 = tmp.tile([128, TOWN], BF16, tag="xbf")
                nc.vector.tensor_mul(t2, t_t[:, c, :], t_t[:, c, :])
                nc.tensor.matmul(ps_var, ones_col_bf, t2,
                                 start=(c == 0), stop=(c == DC - 1))
            sq_sb = tmp.tile([1, TOWN], F32, tag="sqrow")
            nc.scalar.activation(out=sq_sb, in_=ps_var, func=AF.Sqrt,
                                 bias=eps_sb, scale=1.0 / D)
            rstd_sb = tmp.tile([1, TOWN], BF16, tag="mrow")
            nc.vector.reciprocal(out=rstd_sb, in_=sq_sb)
            ps_rb = pp_mm.tile([128, TOWN], F32, tag="mm")
            nc.tensor.matmul(ps_rb, ones_row_bf, rstd_sb, start=True, stop=True)
            for c in range(DC):
                h1 = tmp.tile([128, TOWN], BF16, tag="xbf")
                nc.vector.tensor_tensor(out=h1, in0=t_t[:, c, :], in1=ps_rb,
                                        op=mybir.AluOpType.mult)
                nc.vector.tensor_scalar(out=hn_out[:, c, :], in0=h1,
                                        scalar1=g_ap(c), scalar2=b_ap(c),
                                        op0=OP.mult, op1=OP.add)

        for l in range(L):
            # ---- weights for this layer (prefetchable: bufs rotation) ----
            wq_t = wpool.tile([128, DC, D], BF16, tag="wbig")
            nc.sync.dma_start(out=wq_t, in_=d_wq[l].rearrange("c p n -> p c n"))
            wk_t = wpool.tile([128, DC, D], BF16, tag="wbig")
            nc.sync.dma_start(out=wk_t, in_=d_wk[l].rearrange("c p n -> p c n"))
            wv_t = wpool.tile([128, DC, D], BF16, tag="wbig")
            nc.sync.dma_start(out=wv_t, in_=d_wv[l].rearrange("c p n -> p c n"))

            # ---- LN1 -> hn ----
            hn = acts.tile([128, DC, TOWN], BF16, tag="hn")
            layernorm(lambda c: sb_ln1g[:, l, c:c + 1],
                      lambda c: sb_ln1b[:, l, c:c + 1], hn)

            # ---- pairwise allgather of hn ----
            nc.sync.dma_start(out=d_ccin[l].rearrange("c p t -> p c t"), in_=hn)
            nc.gpsimd.collective_compute(
                "AllGather", mybir.AluOpType.bypass, replica_groups=groups,
                ins=[d_ccin[l][:, :, :]], outs=[d_ccout[l][:, :, :, :]])
            hkv = acts.tile([128, DC, T], BF16, tag="hkv")
            for hf in range(2):
                nc.sync.dma_start(
                    out=hkv[:, :, hf * TOWN:(hf + 1) * TOWN],
                    in_=d_ccout[l][hf].rearrange("c p t -> p c t"))

            # ---- Q (own tokens; overlaps with AG) ----
            qT = acts.tile([128, H // 2, TOWN], BF16, tag="qT")
            for hp in range(H // 2):
                ps = pp_mm.tile([128, TOWN], F32, tag="mm")
                for c in range(DC):
                    nc.tensor.matmul(ps, wq_t[:, c, hp * 128:(hp + 1) * 128],
                                     hn[:, c, :], start=(c == 0), stop=(c == DC - 1))
                nc.scalar.copy(qT[:, hp, :], ps)

            # ---- K (all tokens) ----
            kT = acts.tile([128, H // 2, T], BF16, tag="kT")
            for hp in range(H // 2):
                for hf in range(2):
                    ps = pp_mm.tile([128, TOWN], F32, tag="mm")
                    for c in range(DC):
                        nc.tensor.matmul(ps, wk_t[:, c, hp * 128:(hp + 1) * 128],
                                         hkv[:, c, hf * TOWN:(hf + 1) * TOWN],
                                         start=(c == 0), stop=(c == DC - 1))
                    nc.scalar.copy(kT[:, hp, hf * TOWN:(hf + 1) * TOWN], ps)

            # ---- V (all tokens, token-major, 65-strided with ones col) ----
            v65 = acts.tile([128, 8, H * 65], BF16, tag="v65")
            for tc_ in range(8):
                v3 = v65[:, tc_, :].rearrange("p (h e) -> p h e", e=65)
                for hf in range(2):
                    ps = pp_mm.tile([128, TOWN], F32, tag="mm")
                    for c in range(DC):
                        nc.tensor.matmul(ps, hkv[:, c, tc_ * 128:(tc_ + 1) * 128],
                                         wv_t[:, c, hf * TOWN:(hf + 1) * TOWN],
                                         start=(c == 0), stop=(c == DC - 1))
                    nc.vector.tensor_copy(
                        out=v3[:, hf * 8:(hf + 1) * 8, 0:64],
                        in_=ps.rearrange("p (h e) -> p h e", e=64))
                nc.vector.memset(v3[:, :, 64:65], 1.0)

            # ---- attention, head-pair interleaved (even head on PE rows
            # 0-63, odd head on rows 64-127: the scores matmuls of the two
            # heads use disjoint row-groups and overlap in the array) ----
            attall = acts.tile([128, H // 2, TOWN], BF16, tag="attall")
            for hp in range(H // 2):
                ps_att_a = pp_att.tile([65, TOWN], F32, tag="att")
                ps_att_b = pp_att.tile([65, TOWN], F32, tag="att")
                ps_atts = [ps_att_a, ps_att_b]
                for c in range(8):
                    wss = []
                    for sub in range(2):
                        off = sub * 64
                        ps_s = pp_mm.tile([128, TOWN], F32, tag="mm")
                        nc.tensor.matmul(
                            ps_s, kT[off:off + 64, hp, c * 128:(c + 1) * 128],
                            qT[off:off + 64, hp, :], start=True, stop=True)
                        ws = wsb.tile([128, TOWN], BF16, tag="ws")
                        nc.scalar.activation(out=ws, in_=ps_s, func=AF.Exp,
                                             scale=SCALE)
                        nc.vector.tensor_mul(ws, ws, sb_mask[:, c, :])
                        wss.append(ws)
                    for sub in range(2):
                        h = 2 * hp + sub
                        nc.tensor.matmul(ps_atts[sub],
                                         v65[:, c, h * 65:(h + 1) * 65],
                                         wss[sub], start=(c == 0), stop=(c == 7))
                for sub in range(2):
                    off = sub * 64
                    ps_att = ps_atts[sub]
                    recip = tmp.tile([1, TOWN], BF16, tag="mrow")
                    nc.vector.reciprocal(out=recip, in_=ps_att[64:65, :])
                    ps_r = pp_row.tile([64, TOWN], F32, tag="row")
                    nc.tensor.matmul(ps_r, ones_row_bf[:, 0:64], recip,
                                     start=True, stop=True)
                    att_un = wsb.tile([64, TOWN], BF16, tag="attun")
                    nc.scalar.copy(att_un, ps_att[0:64, :])
                    nc.vector.tensor_tensor(out=attall[off:off + 64, hp, :],
                                            in0=att_un, in1=ps_r,
                                            op=mybir.AluOpType.mult)

            # ---- out-projection + residual ----
            wo_t = wpool.tile([128, DC, D], BF16, tag="wbig")
            nc.sync.dma_start(out=wo_t, in_=d_wo[l].rearrange("c p n -> p c n"))
            for dc in range(DC):
                ps = pp_mm.tile([128, TOWN], F32, tag="mm")
                for hp in range(H // 2):
                    nc.tensor.matmul(ps, wo_t[:, hp, dc * 128:(dc + 1) * 128],
                                     attall[:, hp, :],
                                     start=(hp == 0), stop=(hp == H // 2 - 1))
                nc.vector.scalar_tensor_tensor(
                    out=x_sb[:, dc, :], in0=ps, scalar=sb_bo[:, l, dc:dc + 1],
                    in1=x_sb[:, dc, :], op0=OP.add, op1=OP.add)

            # ---- LN2 -> hn2 ----
            hn2 = acts.tile([128, DC, TOWN], BF16, tag="hn")
            layernorm(lambda c: sb_ln2g[:, l, c:c + 1],
                      lambda c: sb_ln2b[:, l, c:c + 1], hn2)

            # ---- FFN ----
            y2acc = acts.tile([128, DC, TOWN], F32, tag="y2acc")
            for fq in range(FQ):
                w1_t = wpool.tile([128, DC, FF // FQ], BF16, tag="wbig")
                nc.sync.dma_start(
                    out=w1_t,
                    in_=d_w1[l, :, :, fq * (FF // FQ):(fq + 1) * (FF // FQ)]
                    .rearrange("c p n -> p c n"))
                uq = acts.tile([128, 8, TOWN], BF16, tag="uq")
                for fs in range(8):
                    ffc = fq * 8 + fs
                    ps = pp_mm.tile([128, TOWN], F32, tag="mm")
                    for c in range(DC):
                        nc.tensor.matmul(ps, w1_t[:, c, fs * 128:(fs + 1) * 128],
                                         hn2[:, c, :],
                                         start=(c == 0), stop=(c == DC - 1))
                    nc.scalar.activation(out=uq[:, fs, :], in_=ps, func=AF.Relu,
                                         bias=sb_b1[:, l, ffc:ffc + 1])
                w2_t = wpool.tile([128, 8, D], BF16, tag="wbig")
                nc.sync.dma_start(
                    out=w2_t,
                    in_=d_w2[l, fq * 8:(fq + 1) * 8].rearrange("c p n -> p c n"))
                for dc in range(DC):
                    ps = pp_mm.tile([128, TOWN], F32, tag="mm")
                    for fs in range(8):
                        nc.tensor.matmul(ps, w2_t[:, fs, dc * 128:(dc + 1) * 128],
                                         uq[:, fs, :],
                                         start=(fs == 0), stop=(fs == 7))
                    if fq == 0:
                        nc.vector.tensor_scalar(
                            out=y2acc[:, dc, :], in0=ps,
                            scalar1=sb_b2[:, l, dc:dc + 1], scalar2=None,
                            op0=OP.add)
                    else:
                        nc.vector.tensor_add(y2acc[:, dc, :], y2acc[:, dc, :], ps)
            for dc in range(DC):
                nc.vector.tensor_add(x_sb[:, dc, :], x_sb[:, dc, :],
                                     y2acc[:, dc, :])

        # ---- final LN + LM head ----
        xfn = acts.tile([128, DC, TOWN], BF16, tag="hn")
        layernorm(lambda c: sb_lnfg[:, c:c + 1], lambda c: sb_lnfb[:, c:c + 1], xfn)
        for joff, nt in VTILES:
            lmw_t = wpool.tile([128, DC, 512], BF16, tag="wbig")
            nc.sync.dma_start(out=lmw_t[:, :, 0:nt],
                              in_=d_lmw[:, :, joff:joff + nt]
                              .rearrange("c p n -> p c n"))
            for tq in range(TOWN // 128):
                ps = pp_mm.tile([128, TOWN], F32, tag="mm")
                for c in range(DC):
                    nc.tensor.matmul(ps[:, 0:nt],
                                     xfn[:, c, tq * 128:(tq + 1) * 128],
                                     lmw_t[:, c, 0:nt],
                                     start=(c == 0), stop=(c == DC - 1))
                so = stg.tile([128, 512], F32, tag="stage")
                nc.scalar.copy(so[:, 0:nt], ps[:, 0:nt])
                nc.sync.dma_start(out=d_logits[tq, :, joff:joff + nt],
                                  in_=so[:, 0:nt])
        ctx.close()

    nc.compile()
    return nc


def _host_prep(inputs):
    """Build the 8 per-core input maps from the full-model inputs."""
    f32 = lambda a: np.asarray(a, dtype=np.float32)
    tok_emb = f32(inputs["tok_emb"])
    pos_emb = f32(inputs["pos_emb"])
    ctxs = np.asarray(inputs["contexts"]).astype(np.int64)

    x0 = tok_emb[ctxs] + pos_emb[None, :T]  # [B, T, D] f32

    shared = {}
    for name, w in (("wq", inputs["Wq"]), ("wk", inputs["Wk"]), ("wv", inputs["Wv"])):
        w = f32(w).transpose(0, 2, 1, 3).reshape(L, D, D)  # [L, d, h*hs]
        shared[name] = np.ascontiguousarray(w.reshape(L, DC, 128, D)).astype(BF)
    shared["wo"] = np.ascontiguousarray(
        f32(inputs["Wo"]).reshape(L, DC, 128, D)).astype(BF)
    shared["w1"] = np.ascontiguousarray(
        f32(inputs["W1"]).reshape(L, DC, 128, FF)).astype(BF)
    shared["w2"] = np.ascontiguousarray(
        f32(inputs["W2"]).reshape(L, FF // 128, 128, D)).astype(BF)
    shared["lmw"] = np.ascontiguousarray(
        f32(inputs["lm_W"]).reshape(DC, 128, V)).astype(BF)
    for name, key in (("ln1g", "ln1_g"), ("ln1b", "ln1_b"), ("ln2g", "ln2_g"),
                      ("ln2b", "ln2_b"), ("bo", "bo"), ("b2", "b2")):
        shared[name] = np.ascontiguousarray(f32(inputs[key]).reshape(L, DC, 128))
    shared["b1"] = np.ascontiguousarray(f32(inputs["b1"]).reshape(L, FF // 128, 128))
    shared["lnfg"] = np.ascontiguousarray(f32(inputs["lnf_g"]).reshape(DC, 128))
    shared["lnfb"] = np.ascontiguousarray(f32(inputs["lnf_b"]).reshape(DC, 128))

    # masks: kv tokens in global order (pair order == global order);
    # q tokens are the core's own half.  mask[c, i, j] = (c*128+i) <= q_global(j)
    kv_ids = (np.arange(T)).reshape(8, 128)
    masks = {}
    for half in range(2):
        q_ids = half * TOWN + np.arange(TOWN)
        m = (kv_ids[:, :, None] <= q_ids[None, None, :])
        masks[half] = np.ascontiguousarray(m).astype(BF)

    in_maps = []
    for c in range(NCORES):
        b, half = c // 2, c % 2
        xT = x0[b, half * TOWN:(half + 1) * TOWN, :].T  # [D, 512]
        m = dict(shared)
        m["x0"] = np.ascontiguousarray(xT.reshape(DC, 128, TOWN), dtype=np.float32)
        m["mask"] = masks[half]
        in_maps.append(m)
    return in_maps


_LAST_RESULTS = {}


def kernel(**inputs):
    from concourse.bass_utils import run_bass_kernel_spmd

    nc = _build_program()
    in_maps = _host_prep(inputs)
    res = run_bass_kernel_spmd(nc, in_maps, list(range(NCORES)))
    _LAST_RESULTS["res"] = res

    lm_b = np.asarray(inputs["lm_b"], dtype=np.float32)
    logits = np.empty((B * T, V), dtype=np.float32)
    for c in range(NCORES):
        b, half = c // 2, c % 2
        part = res.results[c]["logits"].reshape(TOWN, V)
        logits[b * T + half * TOWN: b * T + (half + 1) * TOWN] = part
    logits += lm_b[None, :]

    # host loss: mean cross-entropy (stable log-softmax in fp64)
    tgt = np.asarray(inputs["targets"]).astype(np.int64).reshape(-1)
    lg = logits.astype(np.float64)
    mx = lg.max(axis=1, keepdims=True)
    lse = mx[:, 0] + np.log(np.exp(lg - mx).sum(axis=1))
    lt = lg[np.arange(B * T), tgt]
    loss = np.float32(-(lt - lse).mean())
    return logits, loss
